# revision 15
# baseline (speedup 1.0000x reference)
"""Trainium2 Bass kernel for nn_DiffusionDecoder (diffusion decoder losses).

Computes (loss_diffusion, loss_species, l_repulsion) from full inputs,
data-parallel over crystals across 8 NeuronCores.

Per-core device program:
  - species head: hidden = silu(h @ W1 + b1) via PE (bf16, hid-partition
    layout), logits per 128-atom tile via PE (atom-partition layout),
    exp/ln on ACT, segmented sum + one-hot pick on DVE.
  - repulsion: per-crystal pairwise wrapped distances in crystal-partition
    layout [128 crystals x 4096 pairs], Gram-form quadratic via fused DVE
    ops, sqrt/square-accumulate tail on ACT.
Host: tables, sharding, final scalar assembly (trivial reductions).
"""
import numpy as np
import ml_dtypes

import concourse.bass as bass
import concourse.bacc as bacc
import concourse.tile as tile
from concourse import mybir
from concourse.bass_utils import run_bass_kernel_spmd

import operator
import concourse.dve_ops as dve_ops
from concourse.dve_ops import DveOp
from concourse.dve_spec import (C0, C1, C2, AluOp, Bin, Spec, Src0, Src1, Zero,
                                lower as _dve_lower, relu as _relu,
                                select as _select, sq as _sq,
                                _has_src1 as _dve_has_src1)
from concourse.dve_uop import DveOpSpec


def _register_dve_op(name, spec):
    """Register a new custom DVE op at runtime (sha computed, not pinned)."""
    if name in dve_ops._SUB_OPCODE_FOR_NAME:
        return next(o for o in dve_ops.OPS if o.name == name)
    row = dve_ops._CUSTOM_DVE_ROW_BASE + len(dve_ops.OPS)
    assert row < 0x20
    dve_ops._SUB_OPCODE_FOR_NAME[name] = row
    shas = {}
    for ver in ("v3", "v4"):
        s = DveOpSpec(name=name, opcode=row, uops=_dve_lower(spec, ver=ver),
                      rd1_en=_dve_has_src1(spec))
        shas[ver] = s.sha(ver)
    op = DveOp(name, spec, subdim=False, uops_sha=shas)
    dve_ops.OPS.append(op)
    dve_ops.CUSTOM_DVE_SPECS[name] = spec
    return op


def _sub(a, b):
    return Bin(AluOp.SUBTRACT, a, b)


def _lt(a, b):
    return Bin(AluOp.IS_LT, a, b)


def _gt(a, b):
    return Bin(AluOp.IS_GT, a, b)


_d = _sub(Src0, Src1)
WRAP_DIFF = _register_dve_op(
    "ANT_WRAP_DIFF",
    Spec(body=_d + _sub(_lt(_d, _sub(Zero, C0)), _gt(_d, C0)),
         reference=lambda in0, in1, s0, s1, imm2: (
             (in0.astype(np.float32) - in1)
             + (((in0.astype(np.float32) - in1) < -s0).astype(np.float32)
                - ((in0.astype(np.float32) - in1) > s0).astype(np.float32)))))
LC2 = _register_dve_op(
    "ANT_LC2",
    Spec(body=Src0 * C0 + Src1 * C1,
         reference=lambda in0, in1, s0, s1, imm2: (
             in0.astype(np.float32) * s0 + in1 * s1)))
SQLC_PLUS = _register_dve_op(
    "ANT_SQLC_PLUS",
    Spec(body=_sq(Src0 + Src1 * C0) + _sq(Src1) * C1,
         reference=lambda in0, in1, s0, s1, imm2: (
             (in0.astype(np.float32) + in1 * s0) ** 2
             + in1.astype(np.float32) ** 2 * s1)))
SQLC2 = _register_dve_op(
    "ANT_SQLC2",
    Spec(body=_sq(Src0 * C0 + Src1 * C1),
         reference=lambda in0, in1, s0, s1, imm2: (
             (in0.astype(np.float32) * s0 + in1 * s1) ** 2)))
ADD_MAX0 = _register_dve_op(
    "ANT_ADD_MAX0",
    Spec(body=_relu(Src0 + Src1),
         reference=lambda in0, in1, s0, s1, imm2: np.maximum(
             in0.astype(np.float32) + in1, 0.0)))


def _rep_tail_ref(in0, in1, s0, s1, imm2):
    a = in0.astype(np.float32)
    b = np.where(a < s1, (s1 - a) ** 2, 0.0).astype(np.float32)
    return b, s0 + b.reshape(b.shape[0], -1).sum(axis=-1, keepdims=True)


REP_TAIL = _register_dve_op(
    "ANT_REP_TAIL",
    Spec(body=_select(_lt(Src0, C1), _sq(_sub(C1, Src0)), Zero),
         accum=operator.add, accum_init=C0,
         reference=_rep_tail_ref))

F32 = mybir.dt.float32
BF16 = mybir.dt.bfloat16
AF = mybir.ActivationFunctionType
OP = mybir.AluOpType

TIMESTEPS = 1000
B = 2048
NPER = 64
N = B * NPER
D = 64            # node dim
H = 128           # hidden dim
C = 100           # species
NCORES = 8
B_LOC = B // NCORES            # 256 crystals / core
N_LOC = N // NCORES            # 16384 atoms / core
NT = N_LOC // 128              # 128 atom tiles / core
NG = 8                         # logits groups
TPG = NT // NG                 # 16 tiles per group
CT = B_LOC // 128              # 2 crystal tiles / core
NPAIR = NPER * NPER            # 4096


def _cosine_schedule(T, s=0.008):
    x = np.linspace(0.0, T, T + 1, dtype=np.float64)
    acp = np.cos(((x / T) + s) / (1.0 + s) * np.pi / 2.0) ** 2
    acp = acp / acp[0]
    betas = np.clip(1.0 - acp[1:] / acp[:-1], 1e-4, 0.999)
    alphas_cumprod = np.cumprod(1.0 - betas)
    return (np.sqrt(alphas_cumprod).astype(np.float32),
            np.sqrt(1.0 - alphas_cumprod).astype(np.float32))


SQRT_ACP, SQRT_OM_ACP = _cosine_schedule(TIMESTEPS)

_COMPILED = {}


def _build_program():
    nc = bacc.Bacc(None, target_bir_lowering=False)

    # ---- per-core external inputs ----
    ht = nc.dram_tensor("ht", [D, N_LOC], BF16, kind="ExternalInput")
    w1 = nc.dram_tensor("w1", [D, H], BF16, kind="ExternalInput")
    w2 = nc.dram_tensor("w2", [H, C], BF16, kind="ExternalInput")
    b1c = nc.dram_tensor("b1c", [H, 1], F32, kind="ExternalInput")
    iotac = nc.dram_tensor("iotac", [128, C], F32, kind="ExternalInput")
    spc = nc.dram_tensor("spc", [128, NT], F32, kind="ExternalInput")
    frac = nc.dram_tensor("frac", [B_LOC, 3 * NPER], F32, kind="ExternalInput")
    nois = nc.dram_tensor("nois", [B_LOC, 3 * NPER], F32, kind="ExternalInput")
    pnoi = nc.dram_tensor("pnoi", [B_LOC, 3 * NPER], F32, kind="ExternalInput")
    # per-crystal scalars, packed [B_LOC, 8]:
    # 0:sa 1:so 2:shift 3:g00 4:g11 5:g22 6:-2g01/g00(neg) 7:-2g02/g00(neg)
    # and a second pack for the rest: 0:-2g12/g11(neg)
    csc = nc.dram_tensor("csc", [B_LOC, 9], F32, kind="ExternalInput")

    out = nc.dram_tensor("out", [128, 12], F32, kind="ExternalOutput")

    with tile.TileContext(nc) as tc:
        with (
            tc.tile_pool(name="const", bufs=1) as cpool,
            tc.tile_pool(name="big", bufs=1) as bpool,
            tc.tile_pool(name="work", bufs=2) as wpool,
            tc.tile_pool(name="qf", bufs=1) as qpool,
            tc.tile_pool(name="psA", bufs=1, space="PSUM") as psA,
            tc.tile_pool(name="psB", bufs=1, space="PSUM") as psB,
        ):
            # ---------------- constants ----------------
            w1t = cpool.tile([D, H], BF16)
            nc.sync.dma_start(w1t[:], w1[:])
            w2t = cpool.tile([H, C], BF16)
            nc.sync.dma_start(w2t[:], w2[:])
            b1t = cpool.tile([H, 1], F32)
            nc.sync.dma_start(b1t[:], b1c[:])
            iot = cpool.tile([128, C], F32)
            nc.sync.dma_start(iot[:], iotac[:])
            spt = cpool.tile([128, NT], F32)
            nc.sync.dma_start(spt[:], spc[:])

            res = cpool.tile([128, 12], F32)
            nc.vector.memset(res[:], 0.0)
            eps8 = cpool.tile([128, 1], F32)
            nc.vector.memset(eps8[:], 1e-8)

            # ---------------- species head ----------------
            hidden = bpool.tile([H, N_LOC], BF16)   # 32KB/partition
            FCH = 2048
            for ch in range(N_LOC // FCH):
                htc = wpool.tile([D, FCH], BF16, tag="htc")
                nc.sync.dma_start(htc[:], ht[:, ch * FCH:(ch + 1) * FCH])
                ps1 = psA.tile([H, FCH], F32, tag="ps1")
                for j in range(FCH // 512):
                    nc.tensor.matmul(
                        ps1[:, j * 512:(j + 1) * 512],
                        w1t[:],
                        htc[:, j * 512:(j + 1) * 512],
                        start=True, stop=True,
                    )
                sg = wpool.tile([H, FCH], F32, tag="sg")
                nc.scalar.activation(sg[:], ps1[:],
                                     AF.Sigmoid, bias=b1t[:, 0:1], scale=1.0)
                # silu(x) = x*sigmoid(x) with x = ps1 + b1
                nc.vector.scalar_tensor_tensor(
                    hidden[:, ch * FCH:(ch + 1) * FCH],
                    ps1[:], b1t[:, 0:1], sg[:], op0=OP.add, op1=OP.mult)

            lncols = cpool.tile([128, NG], F32)
            pkcols = cpool.tile([128, NG], F32)
            for g in range(NG):
                lg = psB.tile([128, TPG, 128], F32, tag="lg")
                for j in range(TPG):
                    at = g * TPG + j
                    nc.tensor.matmul(
                        lg[:, j, 0:C],
                        hidden[:, at * 128:(at + 1) * 128],
                        w2t[:],
                        start=True, stop=True,
                    )
                # E = exp(logits) -> bf16 SBUF
                eg = wpool.tile([128, TPG, C], BF16, tag="eg")
                nc.scalar.activation(eg[:], lg[:, :, 0:C], AF.Exp)
                # per-atom sum over classes
                se = wpool.tile([128, TPG], F32, tag="se")
                nc.vector.tensor_reduce(se[:], eg[:], axis=mybir.AxisListType.X,
                                        op=OP.add)
                # ln(sumexp), accumulate sum over the group's free dim
                lnscr = wpool.tile([128, TPG], F32, tag="lnscr")
                nc.scalar.activation(lnscr[:], se[:], AF.Ln,
                                     accum_out=lncols[:, g:g + 1])
                # one-hot pick of target logit, summed
                msk = wpool.tile([128, TPG, C], F32, tag="msk")
                nc.vector.tensor_tensor(
                    msk[:],
                    iot[:].unsqueeze(1).broadcast_to([128, TPG, C]),
                    spt[:, g * TPG:(g + 1) * TPG].unsqueeze(2).broadcast_to(
                        [128, TPG, C]),
                    op=OP.is_equal,
                )
                pks = wpool.tile([128, TPG, C], BF16, tag="pks")
                nc.vector.scalar_tensor_tensor(
                    pks[:], msk[:], 0.0, lg[:, :, 0:C],
                    op0=OP.bypass, op1=OP.mult,
                    accum_out=pkcols[:, g:g + 1],
                )

            # reduce group columns
            nc.vector.tensor_reduce(res[:, 4:5], lncols[:], axis=mybir.AxisListType.X, op=OP.add)
            nc.vector.tensor_reduce(res[:, 5:6], pkcols[:], axis=mybir.AxisListType.X, op=OP.add)

            # ---------------- repulsion ----------------
            ones_col = None
            for ct in range(CT):
                sl = slice(ct * 128, (ct + 1) * 128)
                fr = wpool.tile([128, 3 * NPER], F32, tag="fr")
                nc.sync.dma_start(fr[:], frac[sl, :])
                no = wpool.tile([128, 3 * NPER], F32, tag="no")
                nc.sync.dma_start(no[:], nois[sl, :])
                pn = wpool.tile([128, 3 * NPER], F32, tag="pn")
                nc.sync.dma_start(pn[:], pnoi[sl, :])
                cs = wpool.tile([128, 9], F32, tag="cs")
                nc.sync.dma_start(cs[:], csc[sl, :])

                sa = cs[:, 0:1]; so = cs[:, 1:2]; isa = cs[:, 2:3]
                r00 = cs[:, 3:4]; r01 = cs[:, 4:5]; r02 = cs[:, 5:6]
                r11 = cs[:, 6:7]; r12 = cs[:, 7:8]; r22sq = cs[:, 8:9]

                # mse partial: sum (pn - no)^2  -> res col 6/7
                m = wpool.tile([128, 3 * NPER], F32, tag="prep", bufs=7)
                nc.vector.tensor_tensor(m[:], pn[:], no[:], op=OP.subtract)
                ms = wpool.tile([128, 3 * NPER], F32, tag="prep", bufs=7)
                from concourse.dve_ops import TENSOR_TENSOR_REDUCE as TTR_OP
                nc.vector._custom_dve(
                    TTR_OP, out=ms[:], in0=m[:], in1=m[:],
                    s0=0.0, s1=1.0, accum_out=res[:, 6 + ct:7 + ct])

                # x_t = sa*frac + so*noise ; wrap
                t1 = wpool.tile([128, 3 * NPER], F32, tag="prep", bufs=7)
                nc.vector.tensor_scalar(t1[:], no[:], so, None, op0=OP.mult)
                xt = wpool.tile([128, 3 * NPER], F32, tag="prep", bufs=7)
                nc.vector.scalar_tensor_tensor(
                    xt[:], fr[:], sa, t1[:], op0=OP.mult, op1=OP.add)
                # wrap xt via int cast + range fixup:
                # u = xt - int(xt) in (-1,1); xtw' = u - 0.5 + (u<0) = frac(xt)-0.5
                xi = wpool.tile([128, 3 * NPER], mybir.dt.int32, tag="prep", bufs=7)
                nc.vector.tensor_copy(xi[:], xt[:])
                xf = wpool.tile([128, 3 * NPER], F32, tag="prep", bufs=7)
                nc.vector.tensor_copy(xf[:], xi[:])
                u1 = wpool.tile([128, 3 * NPER], F32, tag="prep", bufs=7)
                nc.vector.tensor_tensor(u1[:], xt[:], xf[:], op=OP.subtract)
                xtw = wpool.tile([128, 3 * NPER], F32, tag="prep", bufs=7)
                nc.vector.add_range_wrap(xtw[:], u1[:], shift=-0.5,
                                         bound=0.5, period=1.0)
                # pred_x0 = ((xtw'+0.5) - so*pn)/sa ; wrap (mod-1 only)
                t2 = wpool.tile([128, 3 * NPER], F32, tag="prep", bufs=7)
                nc.vector.tensor_scalar(t2[:], pn[:], so, None, op0=OP.mult)
                t3 = wpool.tile([128, 3 * NPER], F32, tag="prep", bufs=7)
                nc.vector.scalar_tensor_tensor(t3[:], xtw[:], 0.5, t2[:],
                                               op0=OP.add, op1=OP.subtract)
                px = wpool.tile([128, 3 * NPER], F32, tag="prep", bufs=7)
                nc.vector.tensor_scalar(px[:], t3[:], isa, None, op0=OP.mult)
                pi = wpool.tile([128, 3 * NPER], mybir.dt.int32, tag="prep", bufs=7)
                nc.vector.tensor_copy(pi[:], px[:])
                pf = wpool.tile([128, 3 * NPER], F32, tag="prep", bufs=7)
                nc.vector.tensor_copy(pf[:], pi[:])
                u2 = wpool.tile([128, 3 * NPER], F32, tag="prep", bufs=7)
                nc.vector.tensor_tensor(u2[:], px[:], pf[:], op=OP.subtract)
                pxw = wpool.tile([128, 3 * NPER], F32, tag="prep", bufs=7)
                nc.vector.add_range_wrap(pxw[:], u2[:], shift=-0.5,
                                         bound=0.5, period=1.0)

                # deinterleave -> xk [128, 64] per component (bf16)
                xs = []
                for k in range(3):
                    xk = wpool.tile([128, NPER], BF16, tag=f"x{k}")
                    nc.vector.tensor_copy(
                        xk[:], pxw[:].rearrange("p (a c) -> p a c", c=3)[:, :, k])
                    xs.append(xk)

                # W streams: wrapped pairwise diffs [128, 4096]
                Ws = []
                for k in range(3):
                    wk = qpool.tile([128, NPAIR], BF16, tag=f"w{k}")
                    nc.vector._custom_dve(
                        WRAP_DIFF,
                        out=wk[:].rearrange("p (a b) -> p a b", b=NPER),
                        in0=xs[k][:].unsqueeze(2).broadcast_to([128, NPER, NPER]),
                        in1=xs[k][:].unsqueeze(1).broadcast_to([128, NPER, NPER]),
                        s0=0.5)
                    Ws.append(wk)
                W0, W1_, W2_ = Ws

                # quadratic form via Cholesky G = R^T R:
                # d2 = (R00 W0 + R01 W1 + R02 W2)^2 + (R11 W1 + R12 W2)^2
                #    + (R22 W2)^2
                lc = qpool.tile([128, NPAIR], BF16, tag="qt", bufs=4)
                nc.vector._custom_dve(LC2, out=lc[:], in0=W0[:], in1=W1_[:],
                                      s0=r00, s1=r01)
                y1s = qpool.tile([128, NPAIR], BF16, tag="qt", bufs=4)
                nc.vector._custom_dve(SQLC_PLUS, out=y1s[:], in0=lc[:],
                                      in1=W2_[:], s0=r02, s1=r22sq)
                y2s = qpool.tile([128, NPAIR], BF16, tag="qt", bufs=4)
                nc.vector._custom_dve(SQLC2, out=y2s[:], in0=W1_[:],
                                      in1=W2_[:], s0=r11, s1=r12)
                d2 = qpool.tile([128, NPAIR], BF16, tag="qt", bufs=4)
                nc.vector._custom_dve(ADD_MAX0, out=d2[:], in0=y1s[:], in1=y2s[:])

                # tail: dist = sqrt(d2 + 1e-8); rep = (0.8-dist)^2 if dist<0.8
                dist = qpool.tile([128, NPAIR], F32, tag="qtail", bufs=2)
                nc.scalar.activation(dist[:], d2[:], AF.Sqrt, bias=eps8[:, 0:1])
                rep = qpool.tile([128, NPAIR], F32, tag="qtail", bufs=2)
                nc.vector._custom_dve(REP_TAIL, out=rep[:], in0=dist[:],
                                      s0=0.0, s1=0.8,
                                      accum_out=res[:, 0 + ct:1 + ct])
                # diag value (pair (0,0)) for correction
                nc.vector.tensor_copy(res[:, 2 + ct:3 + ct], rep[:, 0:1])

            nc.sync.dma_start(out[:], res[:])

    return nc


def _prep_inputs(inputs):
    f32 = np.float32
    frac = np.asarray(inputs["frac_coords"], f32)
    noise = np.asarray(inputs["noise"], f32)
    pn = np.asarray(inputs["pred_noise"], f32)
    h = np.asarray(inputs["h_final"], f32)
    lat = np.asarray(inputs["lattice"], f32)
    W1 = np.asarray(inputs["W1"], f32)
    b1 = np.asarray(inputs["b1"], f32)
    W2 = np.asarray(inputs["W2"], f32)
    b2 = np.asarray(inputs["b2"], f32)
    t = np.asarray(inputs["t"]).astype(np.int64)
    species = np.asarray(inputs["species"]).astype(np.int64)

    if np.any(b2):
        raise NotImplementedError("nonzero b2 not supported by this kernel")

    sa_b = SQRT_ACP[t]
    so_b = SQRT_OM_ACP[t]
    inv_sa_b = (1.0 / sa_b).astype(f32)
    G = np.einsum("bkl,bml->bkm", lat.astype(np.float64),
                  lat.astype(np.float64)).astype(f32)
    G64 = G.astype(np.float64)
    Lc = np.linalg.cholesky(G64)                 # lower: G = Lc Lc^T
    R = np.transpose(Lc, (0, 2, 1))              # upper: G = R^T R
    r00 = R[:, 0, 0]; r01 = R[:, 0, 1]; r02 = R[:, 0, 2]
    r11 = R[:, 1, 1]; r12 = R[:, 1, 2]; r22sq = R[:, 2, 2] ** 2
    csc = np.stack([sa_b, so_b, inv_sa_b, r00, r01, r02, r11, r12, r22sq],
                   axis=1).astype(f32)          # [B, 9]

    hT = np.ascontiguousarray(h.T).astype(ml_dtypes.bfloat16)   # [64, N]
    w1b = W1.astype(ml_dtypes.bfloat16)
    w2b = W2.astype(ml_dtypes.bfloat16)
    b1c = b1.reshape(H, 1).copy()
    iotac = np.broadcast_to(np.arange(C, dtype=f32), (128, C)).copy()
    spc_f = species.astype(f32).reshape(NCORES, NT, 128)        # per core

    frac_c = frac.reshape(B, 3 * NPER)
    nois_c = noise.reshape(B, 3 * NPER)
    pnoi_c = pn.reshape(B, 3 * NPER)

    in_maps = []
    for c in range(NCORES):
        asl = slice(c * N_LOC, (c + 1) * N_LOC)
        bsl = slice(c * B_LOC, (c + 1) * B_LOC)
        in_maps.append({
            "ht": np.ascontiguousarray(hT[:, asl]),
            "w1": w1b, "w2": w2b, "b1c": b1c,
            "iotac": iotac,
            "spc": np.ascontiguousarray(spc_f[c].T),   # [128, NT]
            "frac": np.ascontiguousarray(frac_c[bsl]),
            "nois": np.ascontiguousarray(nois_c[bsl]),
            "pnoi": np.ascontiguousarray(pnoi_c[bsl]),
            "csc": np.ascontiguousarray(csc[bsl]),
        })
    host_b2s = float(b2[species].sum(dtype=np.float64))
    return in_maps, host_b2s


def kernel(**inputs) -> tuple:
    if "prog" not in _COMPILED:
        _COMPILED["prog"] = _build_program()
        _COMPILED["prog"].compile()
    nc = _COMPILED["prog"]
    in_maps, host_b2s = _prep_inputs(inputs)
    res = run_bass_kernel_spmd(nc, in_maps, list(range(NCORES)))
    outs = [r["out"] for r in res.results]

    rep_total = 0.0
    mse_total = 0.0
    lse_total = 0.0
    pick_total = 0.0
    for o in outs:
        o = o.astype(np.float64)
        for ct in range(CT):
            rep_total += (o[:, 0 + ct] - NPER * o[:, 2 + ct]).sum()
            mse_total += o[:, 6 + ct].sum()
        lse_total += o[:, 4].sum()
        pick_total += o[:, 5].sum()

    l_rep = rep_total / NPER / B
    mse = mse_total / (N * 3)
    loss_diffusion = np.float32(mse + 5.0 * l_rep)
    loss_species = np.float32((lse_total - (pick_total + host_b2s)) / N)
    l_repulsion = np.float32(l_rep)
    return (loss_diffusion, loss_species, l_repulsion)


if __name__ == "__main__":
    import reference as ref
    inputs = {k: np.asarray(v) for k, v in ref.setup_inputs().items()}
    got = kernel(**inputs)
    print("kernel:", got)
